# revision 13
# baseline (speedup 1.0000x reference)
"""MDCT kernel for Trainium2 (8 NeuronCores, batch-parallel).

Math: MDCT = TDAC fold + DCT-IV (N = 1024), halving the matmul work vs
the direct 2048x1024 frame matrix:
    out[f, k] = sum_m y[f, m] * D[m, k],   D[m, k] = sqrt(2/N) cos(pi/N (m+.5)(k+.5))
with the folded frame y built from X2 = x.reshape(1024, 1024) rows
(hop = 1024; rows f-1 and f make up frame f, center-padded):
    y[f, 0:512]    = G[f]   (= 0 for f = 1024)
    y[f, 512:1024] = H[f-1] (= 0 for f = 0)
    G[r, j] = -w[1535-j] X2[r, 511-j] - w[1536+j] X2[r, 512+j]
    H[r, j] =  w[j]      X2[r, j]     - w[1023-j] X2[r, 1023-j]

The contraction needs the fold index m on partitions, so the host
uploads X2 pre-transposed (xt = X2.T) plus a partition-reversed copy
(xtr[c] = xt[1023-c]) — the fold reversals become straight reads and no
on-chip transpose is needed.  The fold is 2 tensor_scalar mults
(per-partition f32 window scales, DVE fast mode) + 1 add per chunk.
ht is stored frame-shifted by one so the f-1 alignment is a free-dim
offset.  All fp16 on chip; PSUM accumulates fp32.

Schedule: frame tiles are processed in groups of 3 (one [128,1024]
2-bank PSUM tile each), with matmuls emitted chunk-major: phase c loads
one xt/xtr chunk slice, folds it, and runs that chunk's accumulation
matmul for all tiles of the group.  DMA+fold of phase c+1 are emitted
before the matmuls of phase c so the PE never waits after phase 0.
"""

import numpy as np

import concourse.bass as bass
import concourse.bacc as bacc
import concourse.mybir as mybir
import concourse.tile as tile
from concourse.bass_utils import run_bass_kernel_spmd

B = 8
T = 1 << 20
R = 1024          # rows of X2 per channel (T // hop)
CN = 1024         # row width (hop)
NF = 1025         # output frames
NK = 1024         # output bins
GW = 256          # fold/DMA r-slice width per tile group
F16 = mybir.dt.float16
F32 = mybir.dt.float32

_NC_CACHE = None
_D_CACHE = None
_WF_CACHE = None


def build_nc() -> bass.Bass:
    nc = bacc.Bacc("TRN2", target_bir_lowering=False, debug=False)
    xt = nc.dram_tensor("xt", [CN, R], F16, kind="ExternalInput").ap()
    xtr = nc.dram_tensor("xtr", [CN, R], F16, kind="ExternalInput").ap()
    wf = nc.dram_tensor("wf", [1, 4 * 512], F32, kind="ExternalInput").ap()
    d = nc.dram_tensor("d", [CN, NK], F16, kind="ExternalInput").ap()
    out = nc.dram_tensor("out", [NF, NK], F16, kind="ExternalOutput").ap()

    mul = mybir.AluOpType.mult
    add = mybir.AluOpType.add

    with tile.TileContext(nc) as tc:
        with (
            tc.tile_pool(name="persist", bufs=1) as persist,
            tc.tile_pool(name="tmp", bufs=4) as tmp,
            tc.tile_pool(name="outp", bufs=4) as outp,
            tc.tile_pool(name="mmps", bufs=3, space="PSUM") as mmps,
            tc.tile_pool(name="mmpsl", bufs=1, space="PSUM") as mmpsl,
        ):
            # DCT-IV matrix, 8 row chunks: ds[p, c, k] = d[128c + p, k]
            ds = persist.tile([128, 8, NK], F16)
            d_r = d.rearrange("(c p) k -> p c k", p=128)

            # fold window scales (f32 for tensor_scalar), one per partition:
            # wfp[p, v, c] = wvec_v[128c + p], v = (gA, gB, hA, hB)
            wfp = persist.tile([128, 4, 4], F32)
            nc.scalar.dma_start(wfp[:], wf.rearrange("o (v c p) -> p (o v) c", v=4, p=128))

            # xt/xtr chunks: xts[p, c, r] = xt[128c + p, r]
            xts = persist.tile([128, 8, R], F16)
            xtrs = persist.tile([128, 8, R], F16)
            xt_r = xt.rearrange("(c p) r -> p c r", p=128)
            xtr_r = xtr.rearrange("(c p) r -> p c r", p=128)

            # gt[p, c, f] = G[f, 128c + p]  (f < 1024; col 1024 is zero)
            # ht[p, c, 1 + r] = H[r, 128c + p]  (col 0 is zero)
            gt = persist.tile([128, 4, NF], F16)
            ht = persist.tile([128, 4, NF], F16)
            nc.vector.memset(gt[:, :, 1024:1025], 0.0)
            nc.vector.memset(ht[:, :, 0:1], 0.0)

            # groups of frame tiles; group g folds r in [256g, 256g+256)
            groups = [(0, (0, 1)), (1, (2, 3)), (2, (4, 5)), (3, (6, 7))]

            def prep(g: int, c: int):
                """DMA + fold chunk c's slice for group g."""
                r0 = g * GW
                rw = min(GW, R - r0)
                src = 4 + c if c < 4 else c - 4
                if g == 0:
                    nc.scalar.dma_start(ds[:, c, :], d_r[:, c, :])
                    # lead slice only, so phase-0 matmuls start early
                    nc.sync.dma_start(xts[:, src, 0:GW], xt_r[:, src, 0:GW])
                    nc.sync.dma_start(xtrs[:, src, 0:GW], xtr_r[:, src, 0:GW])
                elif g == 1:
                    # backfill the rest of this chunk pair
                    nc.sync.dma_start(xts[:, src, GW:], xt_r[:, src, GW:])
                    nc.sync.dma_start(xtrs[:, src, GW:], xtr_r[:, src, GW:])
                t1 = tmp.tile([128, GW], F16, tag="t1")
                t2 = tmp.tile([128, GW], F16, tag="t2")
                if c < 4:
                    # G on DVE: gt[:, c, r] = gA*xtr[4+c] + gB*xt[4+c]
                    nc.vector.tensor_scalar(
                        t1[:, :rw], xtrs[:, src, r0:r0 + rw],
                        wfp[:, 0, c:c + 1], None, op0=mul,
                    )
                    nc.vector.tensor_scalar(
                        t2[:, :rw], xts[:, src, r0:r0 + rw],
                        wfp[:, 1, c:c + 1], None, op0=mul,
                    )
                    nc.vector.tensor_tensor(
                        gt[:, c, r0:r0 + rw], t1[:, :rw], t2[:, :rw], op=add
                    )
                else:
                    # H: ht[:, c-4, 1+r] = hA*xt[c-4] + hB*xtr[c-4]
                    nc.vector.tensor_scalar(
                        t1[:, :rw], xts[:, src, r0:r0 + rw],
                        wfp[:, 2, src:src + 1], None, op0=mul,
                    )
                    nc.vector.tensor_scalar(
                        t2[:, :rw], xtrs[:, src, r0:r0 + rw],
                        wfp[:, 3, src:src + 1], None, op0=mul,
                    )
                    nc.vector.tensor_tensor(
                        ht[:, src, 1 + r0:1 + r0 + rw], t1[:, :rw], t2[:, :rw],
                        op=add,
                    )

            phases = [(g, tiles, c) for g, tiles in groups for c in range(8)]
            prep(0, 0)
            prep(0, 1)
            ps = {}
            for idx, (g, tiles, c) in enumerate(phases):
                if idx + 2 < len(phases):
                    prep(phases[idx + 2][0], phases[idx + 2][2])
                if c == 0:
                    for j in tiles:
                        ps[j] = mmps.tile([128, NK], F32, tag="mm", name=f"ps{j}")
                    if g == 3:
                        psl = mmpsl.tile([1, NK], F32, tag="mml", name="psl")
                for j in tiles:
                    f0 = j * 128
                    if c < 4:
                        w = gt[:, c, f0:f0 + 128]
                    else:
                        w = ht[:, c - 4, f0:f0 + 128]
                    nc.tensor.matmul(
                        ps[j][:, 0:512], w, ds[:, c, 0:512],
                        start=(c == 0), stop=(c == 7),
                    )
                    nc.tensor.matmul(
                        ps[j][:, 512:1024], w, ds[:, c, 512:1024],
                        start=(c == 0), stop=(c == 7),
                    )
                if g == 3 and c >= 4:
                    # Last frame (f = 1024): first half of y is zero.
                    wl = ht[:, c - 4, 1024:1025]
                    nc.tensor.matmul(
                        psl[:, 0:512], wl, ds[:, c, 0:512],
                        start=(c == 4), stop=(c == 7),
                    )
                    nc.tensor.matmul(
                        psl[:, 512:1024], wl, ds[:, c, 512:1024],
                        start=(c == 4), stop=(c == 7),
                    )
                if c == 7:
                    for j in tiles:
                        f0 = j * 128
                        ot = outp.tile([128, NK], F16)
                        nc.scalar.copy(ot[:, 0:512], ps[j][:, 0:512])
                        nc.scalar.copy(ot[:, 512:1024], ps[j][:, 512:1024])
                        nc.sync.dma_start(out[f0:f0 + 128, :], ot[:])
                    if g == 3:
                        otl = outp.tile([1, NK], F16, tag="ot_last")
                        nc.scalar.copy(otl[:], psl[:])
                        nc.sync.dma_start(out[1024:1025, :], otl[:])

    return nc


def make_d() -> np.ndarray:
    m = np.arange(CN, dtype=np.float64)[:, None]
    k = np.arange(NK, dtype=np.float64)[None, :]
    d = np.sqrt(2.0 / NK) * np.cos(np.pi / NK * (m + 0.5) * (k + 0.5))
    return d.astype(np.float16)


def make_wf(window: np.ndarray) -> np.ndarray:
    w = window.astype(np.float64)
    j = np.arange(512)
    gA = -w[1535 - j]
    gB = -w[1536 + j]
    hA = w[j]
    hB = -w[1023 - j]
    return np.concatenate([gA, gB, hA, hB]).astype(np.float32)[None, :]


def _get_nc() -> bass.Bass:
    global _NC_CACHE
    if _NC_CACHE is None:
        _NC_CACHE = build_nc()
        _NC_CACHE.compile()
    return _NC_CACHE


def run_spmd(x: np.ndarray, window: np.ndarray, **kwargs):
    """Shard, run on 8 cores, return (stacked output, BassKernelResults)."""
    global _D_CACHE, _WF_CACHE
    if _D_CACHE is None:
        _D_CACHE = make_d()
    if _WF_CACHE is None or _WF_CACHE[0] != window.tobytes():
        _WF_CACHE = (window.tobytes(), make_wf(window))
    wf = _WF_CACHE[1]
    x16 = x.astype(np.float16).reshape(B, R, CN)
    in_maps = []
    for b in range(B):
        xtv = np.ascontiguousarray(x16[b].T)
        xtrv = np.ascontiguousarray(xtv[::-1])
        in_maps.append({"xt": xtv, "xtr": xtrv, "wf": wf, "d": _D_CACHE})
    res = run_bass_kernel_spmd(nc=_get_nc(), in_maps=in_maps,
                               core_ids=list(range(B)), **kwargs)
    out = np.stack(
        [res.results[b]["out"].astype(np.float32) for b in range(B)], axis=0
    )
    return out, res


def kernel(x: np.ndarray, window: np.ndarray) -> np.ndarray:
    out, _ = run_spmd(np.asarray(x), np.asarray(window))
    return out


# revision 14
# speedup vs baseline: 1.1383x; 1.1383x over previous
"""MDCT kernel for Trainium2 (8 NeuronCores, batch-parallel).

Math: MDCT = TDAC fold + DCT-IV (N = 1024), halving the matmul work vs
the direct 2048x1024 frame matrix:
    out[f, k] = sum_m y[f, m] * D[m, k],   D[m, k] = sqrt(2/N) cos(pi/N (m+.5)(k+.5))
with the folded frame y built from X2 = x.reshape(1024, 1024) rows
(hop = 1024; rows f-1 and f make up frame f, center-padded):
    y[f, 0:512]    = G[f]   (= 0 for f = 1024)
    y[f, 512:1024] = H[f-1] (= 0 for f = 0)
    G[r, j] = -w[1535-j] X2[r, 511-j] - w[1536+j] X2[r, 512+j]
    H[r, j] =  w[j]      X2[r, j]     - w[1023-j] X2[r, 1023-j]

The contraction needs the fold index m on partitions, so the host
uploads X2 pre-transposed (xt = X2.T) plus a partition-reversed copy
(xtr[c] = xt[1023-c]) — the fold reversals become straight reads and no
on-chip transpose is needed.  The fold is 2 tensor_scalar mults
(per-partition f32 window scales, DVE fast mode) + 1 add per chunk.
ht is stored frame-shifted by one so the f-1 alignment is a free-dim
offset.  All fp16 on chip; PSUM accumulates fp32.

Schedule: frame tiles are processed in groups of 3 (one [128,1024]
2-bank PSUM tile each), with matmuls emitted chunk-major: phase c loads
one xt/xtr chunk slice, folds it, and runs that chunk's accumulation
matmul for all tiles of the group.  DMA+fold of phase c+1 are emitted
before the matmuls of phase c so the PE never waits after phase 0.
"""

import numpy as np

import concourse.bass as bass
import concourse.bacc as bacc
import concourse.mybir as mybir
import concourse.tile as tile
from concourse.bass_utils import run_bass_kernel_spmd

B = 8
T = 1 << 20
R = 1024          # rows of X2 per channel (T // hop)
CN = 1024         # row width (hop)
NF = 1025         # output frames
NK = 1024         # output bins
GW = 384          # fold/DMA r-slice width per tile group
F16 = mybir.dt.float16
F32 = mybir.dt.float32

_NC_CACHE = None
_D_CACHE = None
_WF_CACHE = None


def build_nc() -> bass.Bass:
    nc = bacc.Bacc("TRN2", target_bir_lowering=False, debug=False)
    xt = nc.dram_tensor("xt", [CN, R], F16, kind="ExternalInput").ap()
    xtr = nc.dram_tensor("xtr", [CN, R], F16, kind="ExternalInput").ap()
    wf = nc.dram_tensor("wf", [1, 4 * 512], F32, kind="ExternalInput").ap()
    d = nc.dram_tensor("d", [CN, NK], F16, kind="ExternalInput").ap()
    out = nc.dram_tensor("out", [NF, NK], F16, kind="ExternalOutput").ap()

    mul = mybir.AluOpType.mult
    add = mybir.AluOpType.add

    with tile.TileContext(nc) as tc:
        with (
            tc.tile_pool(name="persist", bufs=1) as persist,
            tc.tile_pool(name="tmp", bufs=4) as tmp,
            tc.tile_pool(name="outp", bufs=4) as outp,
            tc.tile_pool(name="mmps", bufs=3, space="PSUM") as mmps,
            tc.tile_pool(name="mmpsl", bufs=1, space="PSUM") as mmpsl,
        ):
            # DCT-IV matrix, 8 row chunks: ds[p, c, k] = d[128c + p, k]
            ds = persist.tile([128, 8, NK], F16)
            d_r = d.rearrange("(c p) k -> p c k", p=128)

            # fold window scales (f32 for tensor_scalar), one per partition:
            # wfp[p, v, c] = wvec_v[128c + p], v = (gA, gB, hA, hB)
            wfp = persist.tile([128, 4, 4], F32)
            nc.scalar.dma_start(wfp[:], wf.rearrange("o (v c p) -> p (o v) c", v=4, p=128))

            # xt/xtr chunks: xts[p, c, r] = xt[128c + p, r]
            xts = persist.tile([128, 8, R], F16)
            xtrs = persist.tile([128, 8, R], F16)
            xt_r = xt.rearrange("(c p) r -> p c r", p=128)
            xtr_r = xtr.rearrange("(c p) r -> p c r", p=128)

            # gt[p, c, f] = G[f, 128c + p]  (f < 1024; col 1024 is zero)
            # ht[p, c, 1 + r] = H[r, 128c + p]  (col 0 is zero)
            gt = persist.tile([128, 4, NF], F16)
            ht = persist.tile([128, 4, NF], F16)
            nc.vector.memset(gt[:, :, 1024:1025], 0.0)
            nc.vector.memset(ht[:, :, 0:1], 0.0)

            # groups of frame tiles; group g folds r in [384g, 384g+GW)
            groups = [(0, (0, 1, 2)), (1, (3, 4, 5)), (2, (6, 7))]

            def prep(g: int, c: int):
                """DMA + fold chunk c's slice for group g."""
                r0 = g * GW
                rw = min(GW, R - r0)
                src = 4 + c if c < 4 else c - 4
                if g == 0:
                    nc.scalar.dma_start(ds[:, c, :], d_r[:, c, :])
                    # lead slice only, so phase-0 matmuls start early
                    nc.sync.dma_start(xts[:, src, 0:GW], xt_r[:, src, 0:GW])
                    nc.sync.dma_start(xtrs[:, src, 0:GW], xtr_r[:, src, 0:GW])
                elif g == 1:
                    # backfill the rest of this chunk pair
                    nc.sync.dma_start(xts[:, src, GW:], xt_r[:, src, GW:])
                    nc.sync.dma_start(xtrs[:, src, GW:], xtr_r[:, src, GW:])
                t1 = tmp.tile([128, GW], F16, tag="t1")
                t2 = tmp.tile([128, GW], F16, tag="t2")
                if c < 4:
                    # G on DVE: gt[:, c, r] = gA*xtr[4+c] + gB*xt[4+c]
                    nc.vector.tensor_scalar(
                        t1[:, :rw], xtrs[:, src, r0:r0 + rw],
                        wfp[:, 0, c:c + 1], None, op0=mul,
                    )
                    nc.vector.tensor_scalar(
                        t2[:, :rw], xts[:, src, r0:r0 + rw],
                        wfp[:, 1, c:c + 1], None, op0=mul,
                    )
                    nc.vector.tensor_tensor(
                        gt[:, c, r0:r0 + rw], t1[:, :rw], t2[:, :rw], op=add
                    )
                else:
                    # H: ht[:, c-4, 1+r] = hA*xt[c-4] + hB*xtr[c-4]
                    nc.vector.tensor_scalar(
                        t1[:, :rw], xts[:, src, r0:r0 + rw],
                        wfp[:, 2, src:src + 1], None, op0=mul,
                    )
                    nc.vector.tensor_scalar(
                        t2[:, :rw], xtrs[:, src, r0:r0 + rw],
                        wfp[:, 3, src:src + 1], None, op0=mul,
                    )
                    nc.vector.tensor_tensor(
                        ht[:, src, 1 + r0:1 + r0 + rw], t1[:, :rw], t2[:, :rw],
                        op=add,
                    )

            phases = [(g, tiles, c) for g, tiles in groups for c in range(8)]
            prep(0, 0)
            prep(0, 1)
            ps = {}
            for idx, (g, tiles, c) in enumerate(phases):
                if idx + 2 < len(phases):
                    prep(phases[idx + 2][0], phases[idx + 2][2])
                if c == 0:
                    for j in tiles:
                        ps[j] = mmps.tile([128, NK], F32, tag="mm", name=f"ps{j}")
                    if g == 2:
                        psl = mmpsl.tile([1, NK], F32, tag="mml", name="psl")
                for j in tiles:
                    f0 = j * 128
                    if c < 4:
                        w = gt[:, c, f0:f0 + 128]
                    else:
                        w = ht[:, c - 4, f0:f0 + 128]
                    nc.tensor.matmul(
                        ps[j][:, 0:512], w, ds[:, c, 0:512],
                        start=(c == 0), stop=(c == 7),
                    )
                    nc.tensor.matmul(
                        ps[j][:, 512:1024], w, ds[:, c, 512:1024],
                        start=(c == 0), stop=(c == 7),
                    )
                if g == 2 and c >= 4:
                    # Last frame (f = 1024): first half of y is zero.
                    wl = ht[:, c - 4, 1024:1025]
                    nc.tensor.matmul(
                        psl[:, 0:512], wl, ds[:, c, 0:512],
                        start=(c == 4), stop=(c == 7),
                    )
                    nc.tensor.matmul(
                        psl[:, 512:1024], wl, ds[:, c, 512:1024],
                        start=(c == 4), stop=(c == 7),
                    )
                if c == 7:
                    for j in tiles:
                        f0 = j * 128
                        ot = outp.tile([128, NK], F16)
                        nc.scalar.copy(ot[:, 0:512], ps[j][:, 0:512])
                        nc.scalar.copy(ot[:, 512:1024], ps[j][:, 512:1024])
                        nc.sync.dma_start(out[f0:f0 + 128, :], ot[:])
                    if g == 2:
                        otl = outp.tile([1, NK], F16, tag="ot_last")
                        nc.scalar.copy(otl[:], psl[:])
                        nc.sync.dma_start(out[1024:1025, :], otl[:])

    return nc


def make_d() -> np.ndarray:
    m = np.arange(CN, dtype=np.float64)[:, None]
    k = np.arange(NK, dtype=np.float64)[None, :]
    d = np.sqrt(2.0 / NK) * np.cos(np.pi / NK * (m + 0.5) * (k + 0.5))
    return d.astype(np.float16)


def make_wf(window: np.ndarray) -> np.ndarray:
    w = window.astype(np.float64)
    j = np.arange(512)
    gA = -w[1535 - j]
    gB = -w[1536 + j]
    hA = w[j]
    hB = -w[1023 - j]
    return np.concatenate([gA, gB, hA, hB]).astype(np.float32)[None, :]


def _get_nc() -> bass.Bass:
    global _NC_CACHE
    if _NC_CACHE is None:
        _NC_CACHE = build_nc()
        _NC_CACHE.compile()
    return _NC_CACHE


def run_spmd(x: np.ndarray, window: np.ndarray, **kwargs):
    """Shard, run on 8 cores, return (stacked output, BassKernelResults)."""
    global _D_CACHE, _WF_CACHE
    if _D_CACHE is None:
        _D_CACHE = make_d()
    if _WF_CACHE is None or _WF_CACHE[0] != window.tobytes():
        _WF_CACHE = (window.tobytes(), make_wf(window))
    wf = _WF_CACHE[1]
    x16 = x.astype(np.float16).reshape(B, R, CN)
    in_maps = []
    for b in range(B):
        xtv = np.ascontiguousarray(x16[b].T)
        xtrv = np.ascontiguousarray(xtv[::-1])
        in_maps.append({"xt": xtv, "xtr": xtrv, "wf": wf, "d": _D_CACHE})
    res = run_bass_kernel_spmd(nc=_get_nc(), in_maps=in_maps,
                               core_ids=list(range(B)), **kwargs)
    out = np.stack(
        [res.results[b]["out"].astype(np.float32) for b in range(B)], axis=0
    )
    return out, res


def kernel(x: np.ndarray, window: np.ndarray) -> np.ndarray:
    out, _ = run_spmd(np.asarray(x), np.asarray(window))
    return out
